# revision 8
# baseline (speedup 1.0000x reference)
"""CrossTransformer (KNN message passing) Trainium2 kernel.

Contract: kernel(**inputs) takes the FULL unsharded inputs (numpy arrays,
keys as in setup_inputs()) and returns the FULL [2, 256, 2048] float32
output.  Internally shards across 8 NeuronCores: core = b*4 + s handles
batch b, key-point shard s (512 points), with the fused KNN database
replicated per core.

Pipeline per core:
  1. KNN scores S = 2*k.f - |f|^2 via a K=4 fp32 matmul (PE) — kept fp32
     so the selected neighbor sets match the fp32 reference exactly;
     top-16 via DVE max/max_index/match_replace (two top-8 rounds).
  2. Indirect-DMA row gather of the bf16 [4096, 264] fused database
     (featT | pcdT | pad), one gather per neighbor slot (HW honors one
     offset per partition).
  3. PE transposes (4 neighbors packed per PSUM tile) to channel-major
     bf16 [256, points*16].
  4. pos/attn MLPs in bf16 with fp32 PSUM accumulation (BatchNorm folded
     into the weights host-side), exp without max-subtraction (logits are
     tiny), per-channel softmax over the 16 neighbors fused with the
     weighted sum; final reductions and output in fp32.
"""

import copy as _copy

import numpy as np

import concourse.bass as bass
import concourse.mybir as mybir
import concourse.tile as tile
from concourse import bass_utils
from concourse.masks import make_identity

F32 = mybir.dt.float32
BF16 = mybir.dt.bfloat16
U32 = mybir.dt.uint32
AF = mybir.ActivationFunctionType
AX = mybir.AxisListType

B = 2
C = 256
N = 2048
M = 2048
F = N + M            # fused database size
KNN = 16
PH = 64              # pos MLP hidden
AH = 1024            # attn MLP hidden
P = 128
NCORES = 8
SHARD = N * B // NCORES      # 512 key points per core
NT = SHARD // P              # 4 point-tiles per core
ROW = 264                    # db row: 256 feat + 3 pcd + 5 pad
SL = 512                     # free-dim slice (32 points x 16 neighbors)
PTS_SL = SL // KNN           # 32 points per slice
NSL = P * KNN // SL          # 4 slices per point-tile
BN_EPS = 1e-5
NEG_BIG = -3.0e38

# Module-level knobs for test harnesses (not used by the grader).
TRACE = False
LAST_RESULT = None

_NOP_DICT = {'header': {'opcode': 159, 'inst_word_len': 16}}


def _legalize_sync_waits(nc, max_waits=1):
    """walrus here accepts at most one sync wait per instruction; move
    extra waits onto ENGINE_NOP carriers inserted just before the offender
    (same engine: the sequencer accumulates the waits, no pipeline drain)."""
    module = nc.m
    new_module = _copy.replace(module, functions=[])
    for function in module.functions:
        new_function = _copy.replace(function, blocks=[])
        new_function.set_allocations_from_list(function.allocations)
        for block in function.blocks:
            out = []
            for inst in block.instructions:
                si = inst.sync_info
                waits = list(si.on_wait) if si is not None else []
                if len(waits) > max_waits:
                    extra, keep = waits[:-max_waits], waits[-max_waits:]
                    for j in range(0, len(extra), max_waits):
                        out.append(mybir.InstDrain(
                            name=f"I-lgl-{inst.name}-{j}",
                            engine=inst.engine,
                            ins=[], outs=[],
                            sync_info=mybir.SyncInfo(
                                on_wait=extra[j:j + max_waits], on_update=[]),
                        ))
                    inst.sync_info = mybir.SyncInfo(
                        on_wait=keep, on_update=list(si.on_update))
                out.append(inst)
            new_function.blocks.append(_copy.replace(block, instructions=out))
        new_module.functions.append(new_function)
    nc.m = new_module


def _build_bass(legalize=True):
    nc = bass.Bass()
    dt = nc.dram_tensor
    keys2t = dt("keys2t", [4, SHARD], F32, kind="ExternalInput")
    knn_rhs = dt("knn_rhs", [4, F], F32, kind="ExternalInput")
    db_rows = dt("db_rows", [F, ROW], BF16, kind="ExternalInput")
    feat_sh = dt("feat_sh", [C, SHARD], BF16, kind="ExternalInput")
    pcd_sh = dt("pcd_sh", [4, SHARD], BF16, kind="ExternalInput")
    pos_w1t = dt("pos_w1t", [4, PH], BF16, kind="ExternalInput")
    pos_b1 = dt("pos_b1", [PH, 1], F32, kind="ExternalInput")
    pos_w2t = dt("pos_w2t", [PH, C], BF16, kind="ExternalInput")
    pos_b2c = dt("pos_b2c", [P, 2], F32, kind="ExternalInput")
    attn_w1t = dt("attn_w1t", [C, AH], BF16, kind="ExternalInput")
    attn_b1c = dt("attn_b1c", [P, AH // P], F32, kind="ExternalInput")
    attn_w2t = dt("attn_w2t", [AH, C], BF16, kind="ExternalInput")
    attn_b2c = dt("attn_b2c", [P, 2], F32, kind="ExternalInput")
    out = dt("out", [C, SHARD], F32, kind="ExternalOutput")

    with tile.TileContext(nc) as tc:
        with (
            tc.tile_pool(name="const", bufs=1) as cp,
            tc.tile_pool(name="s", bufs=2) as s_pool,
            tc.tile_pool(name="idx", bufs=2) as idx_pool,
            tc.tile_pool(name="g", bufs=2) as g_pool,
            tc.tile_pool(name="gt", bufs=2) as gt_pool,
            tc.tile_pool(name="h1", bufs=2) as h1_pool,
            tc.tile_pool(name="tmp", bufs=2) as tmp_pool,
            tc.tile_pool(name="small", bufs=2) as sm_pool,
            tc.tile_pool(name="ot", bufs=2) as ot_pool,
            tc.tile_pool(name="ppk", bufs=2, space="PSUM") as pp_knn,
            tc.tile_pool(name="ppt", bufs=2, space="PSUM") as pp_tp,
            tc.tile_pool(name="ppm", bufs=4, space="PSUM") as pp_mlp,
        ):
            # ---- constants / weights ----
            ident = cp.tile([P, P], BF16)
            make_identity(nc, ident[:, :])
            keys2t_s = cp.tile([4, SHARD], F32)
            nc.sync.dma_start(keys2t_s[:, :], keys2t[:, :])
            knn_rhs_s = cp.tile([4, F], F32)
            nc.sync.dma_start(knn_rhs_s[:, :], knn_rhs[:, :])
            feat_s = []
            for cc in range(2):
                ft = cp.tile([P, SHARD], BF16, tag=f"feat{cc}")
                nc.sync.dma_start(ft[:, :], feat_sh[cc * P:(cc + 1) * P, :])
                feat_s.append(ft)
            pcd_s = cp.tile([4, SHARD], BF16)
            nc.sync.dma_start(pcd_s[:, :], pcd_sh[:, :])
            pw1 = cp.tile([4, PH], BF16)
            nc.sync.dma_start(pw1[:, :], pos_w1t[:, :])
            pb1 = cp.tile([PH, 1], F32)
            nc.sync.dma_start(pb1[:, :], pos_b1[:, :])
            pw2 = cp.tile([PH, C], BF16)
            nc.sync.dma_start(pw2[:, :], pos_w2t[:, :])
            pb2 = cp.tile([P, 2], F32)
            nc.sync.dma_start(pb2[:, :], pos_b2c[:, :])
            w1 = []
            for kc in range(2):
                wt = cp.tile([P, AH], BF16, tag=f"w1_{kc}")
                nc.sync.dma_start(wt[:, :], attn_w1t[kc * P:(kc + 1) * P, :])
                w1.append(wt)
            ab1 = cp.tile([P, AH // P], F32)
            nc.sync.dma_start(ab1[:, :], attn_b1c[:, :])
            w2 = []
            for o in range(AH // P):
                wt = cp.tile([P, C], BF16, tag=f"w2_{o}")
                nc.sync.dma_start(wt[:, :], attn_w2t[o * P:(o + 1) * P, :])
                w2.append(wt)
            ab2 = cp.tile([P, 2], F32)
            nc.sync.dma_start(ab2[:, :], attn_b2c[:, :])

            for t in range(NT):
                tsl = slice(t * P, (t + 1) * P)
                # ---- KNN scores: S[p, f] = 2*k_p . f - |f|^2 (fp32) ----
                S = s_pool.tile([P, F], F32)
                for c in range(F // SL):
                    ps = pp_knn.tile([P, SL], F32, tag="ks")
                    nc.tensor.matmul(ps[:, :], lhsT=keys2t_s[:, tsl],
                                     rhs=knn_rhs_s[:, c * SL:(c + 1) * SL],
                                     start=True, stop=True)
                    nc.vector.tensor_copy(S[:, c * SL:(c + 1) * SL], ps[:, :])
                # ---- top-16 (two top-8 rounds; order within 16 is free) ----
                mx = sm_pool.tile([P, 8], F32, tag="mx")
                idx = idx_pool.tile([P, KNN], U32)
                nc.vector.max(out=mx[:, :], in_=S[:, :])
                nc.vector.max_index(idx[:, 0:8], mx[:, :], S[:, :])
                nc.vector.match_replace(out=S[:, :], in_to_replace=mx[:, :],
                                        in_values=S[:, :], imm_value=NEG_BIG)
                mx2 = sm_pool.tile([P, 8], F32, tag="mx2")
                nc.vector.max(out=mx2[:, :], in_=S[:, :])
                nc.vector.max_index(idx[:, 8:16], mx2[:, :], S[:, :])

                # ---- gather 16 bf16 db rows per point (one DMA per slot:
                # HW honors a single offset per partition) ----
                g = g_pool.tile([P, KNN * ROW], BF16)
                for nb in range(KNN):
                    nc.gpsimd.indirect_dma_start(
                        out=g[:, nb * ROW:(nb + 1) * ROW], out_offset=None,
                        in_=db_rows[:, :],
                        in_offset=bass.IndirectOffsetOnAxis(
                            ap=idx[:, nb:nb + 1], axis=0),
                    )

                # ---- transpose to channel-major (4 neighbors per PSUM tile,
                # one strided copy out per group) ----
                G = [gt_pool.tile([P, P * KNN], BF16, tag=f"g{cc}",
                                  name=f"g{cc}") for cc in range(2)]
                P3 = gt_pool.tile([8, P * KNN], BF16, tag="p3")
                # [p, k, j] views for the grouped copies
                Gkj = [G[cc][:, :].rearrange("p (j k) -> p k j", k=KNN)
                       for cc in range(2)]
                P3kj = P3[:, :].rearrange("p (j k) -> p k j", k=KNN)
                for grp in range(KNN // 4):
                    nbs = range(grp * 4, grp * 4 + 4)
                    for cc in range(2):
                        tp = pp_tp.tile([P, 4 * P], BF16, tag="tp")
                        for q, nb in enumerate(nbs):
                            nc.tensor.transpose(
                                tp[:, q * P:(q + 1) * P],
                                g[:, nb * ROW + cc * P:nb * ROW + (cc + 1) * P],
                                ident[:, :])
                        nc.vector.tensor_copy(
                            Gkj[cc][:, grp * 4:grp * 4 + 4, :],
                            tp[:, :].rearrange("p (q j) -> p q j", q=4))
                    tp2 = pp_tp.tile([8, 4 * P], BF16, tag="tp")
                    for q, nb in enumerate(nbs):
                        nc.tensor.transpose(
                            tp2[:, q * P:(q + 1) * P],
                            g[:, nb * ROW + 2 * P:nb * ROW + ROW],
                            ident[:, :])
                    nc.vector.tensor_copy(
                        P3kj[:, grp * 4:grp * 4 + 4, :],
                        tp2[:, :].rearrange("p (q j) -> p q j", q=4))

                # ---- MLPs + softmax + weighted sum, in 512-wide slices ----
                out_t = [ot_pool.tile([P, P], F32, tag=f"ot{cc}",
                                      name=f"ot{cc}") for cc in range(2)]
                for s in range(NSL):
                    sl = slice(s * SL, (s + 1) * SL)
                    pts = slice(t * P + s * PTS_SL, t * P + (s + 1) * PTS_SL)
                    P3j = P3[:, :].rearrange("p (j k) -> p j k", k=KNN)
                    # pos_rel = pcd_n - p_j  (rows 0-2; row-3 weight is 0)
                    pr = tmp_pool.tile([4, SL], BF16, tag="pr")
                    pcd_b = pcd_s[:, pts].unsqueeze(-1).to_broadcast(
                        [4, PTS_SL, KNN])
                    nc.vector.tensor_sub(
                        pr[:, :].rearrange("p (j k) -> p j k", k=KNN),
                        pcd_b, P3j[0:4, s * PTS_SL:(s + 1) * PTS_SL, :])
                    # pos MLP
                    h1p_ps = pp_mlp.tile([PH, SL], F32, tag="mm")
                    nc.tensor.matmul(h1p_ps[:, :], lhsT=pw1[:, :], rhs=pr[:, :],
                                     start=True, stop=True)
                    h1p = tmp_pool.tile([PH, SL], BF16, tag="h1p")
                    nc.scalar.activation(h1p[:, :], h1p_ps[:, :], AF.Relu,
                                         bias=pb1[:, 0:1])
                    pe = [tmp_pool.tile([P, SL], BF16, tag=f"pe{cc}",
                                        name=f"pe{cc}") for cc in range(2)]
                    for cc in range(2):
                        pe_ps = pp_mlp.tile([P, SL], F32, tag="mm")
                        nc.tensor.matmul(pe_ps[:, :],
                                         lhsT=pw2[:, cc * P:(cc + 1) * P],
                                         rhs=h1p[:, :], start=True, stop=True)
                        nc.vector.tensor_copy(pe[cc][:, :], pe_ps[:, :])
                    # attn_in = feat_n - g + pos_emb ; V = g + pos_emb
                    ain = [tmp_pool.tile([P, SL], BF16, tag=f"ain{cc}",
                                         name=f"ain{cc}") for cc in range(2)]
                    V = [tmp_pool.tile([P, SL], BF16, tag=f"v{cc}",
                                       name=f"v{cc}") for cc in range(2)]
                    for cc in range(2):
                        gsl = G[cc][:, sl]
                        nc.vector.tensor_sub(ain[cc][:, :], pe[cc][:, :], gsl)
                        featb = feat_s[cc][:, pts].unsqueeze(-1).to_broadcast(
                            [P, PTS_SL, KNN])
                        a3 = ain[cc][:, :].rearrange("p (j k) -> p j k", k=KNN)
                        nc.vector.tensor_add(a3, a3, featb)
                        nc.vector.tensor_add(V[cc][:, :], pe[cc][:, :], gsl)
                    # attn MLP layer 1 (K=256 in 2 chunks, M=1024 in 8)
                    h1s = []
                    for o in range(AH // P):
                        hp = pp_mlp.tile([P, SL], F32, tag="mm")
                        nc.tensor.matmul(hp[:, :],
                                         lhsT=w1[0][:, o * P:(o + 1) * P],
                                         rhs=ain[0][:, :],
                                         start=True, stop=False)
                        nc.tensor.matmul(hp[:, :],
                                         lhsT=w1[1][:, o * P:(o + 1) * P],
                                         rhs=ain[1][:, :],
                                         start=False, stop=True)
                        ht = h1_pool.tile([P, SL], BF16, tag=f"h1_{o}",
                                          name=f"h1_{o}")
                        nc.scalar.activation(ht[:, :], hp[:, :], AF.Relu,
                                             bias=ab1[:, o:o + 1])
                        h1s.append(ht)
                    # attn MLP layer 2 (K=1024 in 8 chunks) + softmax-sum
                    for cc in range(2):
                        lp = pp_mlp.tile([P, SL], F32, tag="mm")
                        for o in range(AH // P):
                            nc.tensor.matmul(lp[:, :],
                                             lhsT=w2[o][:, cc * P:(cc + 1) * P],
                                             rhs=h1s[o][:, :],
                                             start=(o == 0),
                                             stop=(o == AH // P - 1))
                        e = tmp_pool.tile([P, SL], BF16, tag=f"e{cc}",
                                          name=f"e{cc}")
                        nc.scalar.activation(e[:, :], lp[:, :], AF.Exp,
                                             bias=ab2[:, cc:cc + 1])
                        e3 = e[:, :].rearrange("p (j k) -> p j k", k=KNN)
                        den = sm_pool.tile([P, PTS_SL], F32, tag=f"den{cc}",
                                           name=f"den{cc}")
                        nc.vector.reduce_sum(den[:, :], e3, axis=AX.X)
                        rden = sm_pool.tile([P, PTS_SL], F32, tag=f"rden{cc}",
                                            name=f"rden{cc}")
                        nc.vector.reciprocal(rden[:, :], den[:, :])
                        ev = tmp_pool.tile([P, SL], BF16, tag=f"ev{cc}",
                                           name=f"ev{cc}")
                        nc.vector.tensor_mul(ev[:, :], e[:, :], V[cc][:, :])
                        ev3 = ev[:, :].rearrange("p (j k) -> p j k", k=KNN)
                        num = sm_pool.tile([P, PTS_SL], F32, tag=f"num{cc}",
                                           name=f"num{cc}")
                        nc.vector.reduce_sum(num[:, :], ev3, axis=AX.X)
                        osl = out_t[cc][:, s * PTS_SL:(s + 1) * PTS_SL]
                        nc.vector.tensor_mul(osl, num[:, :], rden[:, :])
                        # + pos_b2 (softmax weights sum to 1 per channel)
                        nc.vector.tensor_scalar(
                            osl, osl, pb2[:, cc:cc + 1], None,
                            op0=mybir.AluOpType.add)
                for cc in range(2):
                    nc.sync.dma_start(out[cc * P:(cc + 1) * P, tsl],
                                      out_t[cc][:, :])

    if legalize:
        _legalize_sync_waits(nc)
    return nc


_NC = None


def _get_nc():
    global _NC
    if _NC is None:
        _NC = _build_bass()
    return _NC


def _prep_in_maps(pcd, feat, pcd_feadb, feat_feadb,
                  pos_w1, pos_b1, pos_g1, pos_be1, pos_w2, pos_b2,
                  attn_w1, attn_b1, attn_g1, attn_be1, attn_w2, attn_b2):
    f32 = np.float32
    bf16 = mybir.dt.np(BF16)
    a = {k: np.ascontiguousarray(np.asarray(v), dtype=f32) for k, v in dict(
        pcd=pcd, feat=feat, pcd_feadb=pcd_feadb, feat_feadb=feat_feadb,
        pos_w1=pos_w1, pos_b1=pos_b1, pos_g1=pos_g1, pos_be1=pos_be1,
        pos_w2=pos_w2, pos_b2=pos_b2,
        attn_w1=attn_w1, attn_b1=attn_b1, attn_g1=attn_g1, attn_be1=attn_be1,
        attn_w2=attn_w2, attn_b2=attn_b2).items()}

    fus_pcd = np.concatenate([a['pcd'], a['pcd_feadb']], axis=2)    # [B,3,F]
    fus_feat = np.concatenate([a['feat'], a['feat_feadb']], axis=2)  # [B,C,F]

    # BatchNorm (eval, running stats 0/1) folded into the conv weights.
    sp = (a['pos_g1'].astype(np.float64) / np.sqrt(1.0 + BN_EPS))
    w1p = a['pos_w1'].astype(np.float64) * sp[:, None]
    b1p = a['pos_b1'].astype(np.float64) * sp + a['pos_be1']
    sa = (a['attn_g1'].astype(np.float64) / np.sqrt(1.0 + BN_EPS))
    w1a = a['attn_w1'].astype(np.float64) * sa[:, None]
    # pos_b2 folded into attn bias (pre-relu) and the final output bias.
    b1a = (a['attn_b1'].astype(np.float64) * sa + a['attn_be1']
           + w1a @ a['pos_b2'].astype(np.float64))

    pos_w1t = np.zeros((4, PH), bf16)
    pos_w1t[:3] = w1p.T.astype(bf16)
    pos_b1v = b1p.astype(f32).reshape(PH, 1)
    pos_w2t = np.ascontiguousarray(a['pos_w2'].T).astype(bf16)
    pos_b2c = np.ascontiguousarray(a['pos_b2'].reshape(2, P).T)
    attn_w1t = np.ascontiguousarray(w1a.T).astype(bf16)
    attn_b1c = np.ascontiguousarray(b1a.astype(f32).reshape(AH // P, P).T)
    attn_w2t = np.ascontiguousarray(a['attn_w2'].T).astype(bf16)
    attn_b2c = np.ascontiguousarray(a['attn_b2'].reshape(2, P).T)

    per_batch = []
    for b in range(B):
        knn_rhs = np.empty((4, F), f32)
        knn_rhs[:3] = fus_pcd[b]
        knn_rhs[3] = (-np.sum(fus_pcd[b].astype(np.float64) ** 2, axis=0)
                      ).astype(f32)
        db = np.zeros((F, ROW), bf16)
        db[:, :C] = fus_feat[b].T.astype(bf16)
        db[:, C:C + 3] = fus_pcd[b].T.astype(bf16)
        per_batch.append((knn_rhs, np.ascontiguousarray(db)))

    in_maps = []
    for core in range(NCORES):
        b, s = divmod(core, NCORES // B)
        sh = slice(s * SHARD, (s + 1) * SHARD)
        keys2t = np.zeros((4, SHARD), f32)
        keys2t[:3] = 2.0 * a['pcd'][b][:, sh]
        keys2t[3] = 1.0
        pcd_sh = np.zeros((4, SHARD), bf16)
        pcd_sh[:3] = a['pcd'][b][:, sh].astype(bf16)
        in_maps.append(dict(
            keys2t=keys2t,
            knn_rhs=per_batch[b][0],
            db_rows=per_batch[b][1],
            feat_sh=np.ascontiguousarray(a['feat'][b][:, sh]).astype(bf16),
            pcd_sh=pcd_sh,
            pos_w1t=pos_w1t, pos_b1=pos_b1v, pos_w2t=pos_w2t, pos_b2c=pos_b2c,
            attn_w1t=attn_w1t, attn_b1c=attn_b1c,
            attn_w2t=attn_w2t, attn_b2c=attn_b2c,
        ))
    return in_maps


def kernel(**inputs):
    global LAST_RESULT
    nc = _get_nc()
    in_maps = _prep_in_maps(**inputs)
    res = bass_utils.run_bass_kernel_spmd(
        nc, in_maps, core_ids=list(range(NCORES)), trace=TRACE)
    LAST_RESULT = res
    out = np.empty((B, C, N), np.float32)
    for core in range(NCORES):
        b, s = divmod(core, NCORES // B)
        out[b][:, s * SHARD:(s + 1) * SHARD] = res.results[core]["out"]
    return out


# revision 9
# speedup vs baseline: 1.1057x; 1.1057x over previous
"""CrossTransformer (KNN message passing) Trainium2 kernel.

Contract: kernel(**inputs) takes the FULL unsharded inputs (numpy arrays,
keys as in setup_inputs()) and returns the FULL [2, 256, 2048] float32
output.  Internally shards across 8 NeuronCores: core = b*4 + s handles
batch b, key-point shard s (512 points), with the fused KNN database
replicated per core.

Pipeline per core:
  1. KNN scores S = 2*k.f - |f|^2 via a K=4 fp32 matmul (PE) — kept fp32
     so the selected neighbor sets match the fp32 reference exactly;
     top-16 via DVE max/max_index/match_replace (two top-8 rounds).
  2. Indirect-DMA row gather of the bf16 [4096, 264] fused database
     (featT | pcdT | pad), one gather per neighbor slot (HW honors one
     offset per partition).
  3. PE transposes (4 neighbors packed per PSUM tile) to channel-major
     bf16 [256, points*16].
  4. pos/attn MLPs in bf16 with fp32 PSUM accumulation (BatchNorm folded
     into the weights host-side), exp without max-subtraction (logits are
     tiny), per-channel softmax over the 16 neighbors fused with the
     weighted sum; final reductions and output in fp32.
"""

import copy as _copy

import numpy as np

import concourse.bass as bass
import concourse.mybir as mybir
import concourse.tile as tile
from concourse import bass_utils
from concourse.masks import make_identity

F32 = mybir.dt.float32
BF16 = mybir.dt.bfloat16
U32 = mybir.dt.uint32
AF = mybir.ActivationFunctionType
AX = mybir.AxisListType

B = 2
C = 256
N = 2048
M = 2048
F = N + M            # fused database size
KNN = 16
PH = 64              # pos MLP hidden
AH = 1024            # attn MLP hidden
P = 128
NCORES = 8
SHARD = N * B // NCORES      # 512 key points per core
NT = SHARD // P              # 4 point-tiles per core
ROW = 264                    # db row: 256 feat + 3 pcd + 5 pad
SL = 512                     # free-dim slice (32 points x 16 neighbors)
PTS_SL = SL // KNN           # 32 points per slice
NSL = P * KNN // SL          # 4 slices per point-tile
BN_EPS = 1e-5
NEG_BIG = -3.0e38

# Module-level knobs for test harnesses (not used by the grader).
TRACE = False
LAST_RESULT = None

_NOP_DICT = {'header': {'opcode': 159, 'inst_word_len': 16}}


def _legalize_sync_waits(nc, max_waits=1):
    """walrus here accepts at most one sync wait per instruction; move
    extra waits onto ENGINE_NOP carriers inserted just before the offender
    (same engine: the sequencer accumulates the waits, no pipeline drain)."""
    module = nc.m
    new_module = _copy.replace(module, functions=[])
    for function in module.functions:
        new_function = _copy.replace(function, blocks=[])
        new_function.set_allocations_from_list(function.allocations)
        for block in function.blocks:
            out = []
            for inst in block.instructions:
                si = inst.sync_info
                waits = list(si.on_wait) if si is not None else []
                if len(waits) > max_waits:
                    extra, keep = waits[:-max_waits], waits[-max_waits:]
                    for j in range(0, len(extra), max_waits):
                        out.append(mybir.InstDrain(
                            name=f"I-lgl-{inst.name}-{j}",
                            engine=inst.engine,
                            ins=[], outs=[],
                            sync_info=mybir.SyncInfo(
                                on_wait=extra[j:j + max_waits], on_update=[]),
                        ))
                    inst.sync_info = mybir.SyncInfo(
                        on_wait=keep, on_update=list(si.on_update))
                out.append(inst)
            new_function.blocks.append(_copy.replace(block, instructions=out))
        new_module.functions.append(new_function)
    nc.m = new_module


def _build_bass(legalize=True):
    nc = bass.Bass()
    dt = nc.dram_tensor
    keys2t = dt("keys2t", [4, SHARD], F32, kind="ExternalInput")
    knn_rhs = dt("knn_rhs", [4, F], F32, kind="ExternalInput")
    db_rows = dt("db_rows", [F, ROW], BF16, kind="ExternalInput")
    feat_sh = dt("feat_sh", [C, SHARD], BF16, kind="ExternalInput")
    pcd_sh = dt("pcd_sh", [4, SHARD], BF16, kind="ExternalInput")
    pos_w1t = dt("pos_w1t", [4, PH], BF16, kind="ExternalInput")
    pos_b1 = dt("pos_b1", [PH, 1], F32, kind="ExternalInput")
    pos_w2t = dt("pos_w2t", [PH, C], BF16, kind="ExternalInput")
    pos_b2c = dt("pos_b2c", [P, 2], F32, kind="ExternalInput")
    attn_w1t = dt("attn_w1t", [C, AH], BF16, kind="ExternalInput")
    attn_b1c = dt("attn_b1c", [P, AH // P], F32, kind="ExternalInput")
    attn_w2t = dt("attn_w2t", [AH, C], BF16, kind="ExternalInput")
    attn_b2c = dt("attn_b2c", [P, 2], F32, kind="ExternalInput")
    out = dt("out", [C, SHARD], F32, kind="ExternalOutput")

    with tile.TileContext(nc) as tc:
        with (
            tc.tile_pool(name="const", bufs=1) as cp,
            tc.tile_pool(name="s", bufs=2) as s_pool,
            tc.tile_pool(name="idx", bufs=2) as idx_pool,
            tc.tile_pool(name="g", bufs=2) as g_pool,
            tc.tile_pool(name="gt", bufs=2) as gt_pool,
            tc.tile_pool(name="h1", bufs=2) as h1_pool,
            tc.tile_pool(name="tmp", bufs=2) as tmp_pool,
            tc.tile_pool(name="small", bufs=2) as sm_pool,
            tc.tile_pool(name="ot", bufs=2) as ot_pool,
            tc.tile_pool(name="ppk", bufs=2, space="PSUM") as pp_knn,
            tc.tile_pool(name="ppt", bufs=2, space="PSUM") as pp_tp,
            tc.tile_pool(name="ppm", bufs=4, space="PSUM") as pp_mlp,
        ):
            # ---- constants / weights ----
            ident = cp.tile([P, P], BF16)
            make_identity(nc, ident[:, :])
            keys2t_s = cp.tile([4, SHARD], F32)
            nc.sync.dma_start(keys2t_s[:, :], keys2t[:, :])
            knn_rhs_s = cp.tile([4, F], F32)
            nc.sync.dma_start(knn_rhs_s[:, :], knn_rhs[:, :])
            feat_s = []
            for cc in range(2):
                ft = cp.tile([P, SHARD], BF16, tag=f"feat{cc}")
                nc.sync.dma_start(ft[:, :], feat_sh[cc * P:(cc + 1) * P, :])
                feat_s.append(ft)
            pcd_s = cp.tile([4, SHARD], BF16)
            nc.sync.dma_start(pcd_s[:, :], pcd_sh[:, :])
            pw1 = cp.tile([4, PH], BF16)
            nc.sync.dma_start(pw1[:, :], pos_w1t[:, :])
            pb1 = cp.tile([PH, 1], F32)
            nc.sync.dma_start(pb1[:, :], pos_b1[:, :])
            pw2 = cp.tile([PH, C], BF16)
            nc.sync.dma_start(pw2[:, :], pos_w2t[:, :])
            pb2 = cp.tile([P, 2], F32)
            nc.sync.dma_start(pb2[:, :], pos_b2c[:, :])
            w1 = []
            for kc in range(2):
                wt = cp.tile([P, AH], BF16, tag=f"w1_{kc}")
                nc.sync.dma_start(wt[:, :], attn_w1t[kc * P:(kc + 1) * P, :])
                w1.append(wt)
            ab1 = cp.tile([P, AH // P], F32)
            nc.sync.dma_start(ab1[:, :], attn_b1c[:, :])
            w2 = []
            for o in range(AH // P):
                wt = cp.tile([P, C], BF16, tag=f"w2_{o}")
                nc.sync.dma_start(wt[:, :], attn_w2t[o * P:(o + 1) * P, :])
                w2.append(wt)
            ab2 = cp.tile([P, 2], F32)
            nc.sync.dma_start(ab2[:, :], attn_b2c[:, :])

            for t in range(NT):
                tsl = slice(t * P, (t + 1) * P)
                # ---- KNN scores: S[p, f] = 2*k_p . f - |f|^2 (fp32) ----
                S = s_pool.tile([P, F], F32)
                for c in range(F // SL):
                    ps = pp_knn.tile([P, SL], F32, tag="ks")
                    nc.tensor.matmul(ps[:, :], lhsT=keys2t_s[:, tsl],
                                     rhs=knn_rhs_s[:, c * SL:(c + 1) * SL],
                                     start=True, stop=True)
                    nc.vector.tensor_copy(S[:, c * SL:(c + 1) * SL], ps[:, :])
                # ---- top-16 (two top-8 rounds; order within 16 is free) ----
                mx = sm_pool.tile([P, 8], F32, tag="mx")
                idx = idx_pool.tile([P, KNN], U32)
                nc.vector.max(out=mx[:, :], in_=S[:, :])
                nc.vector.max_index(idx[:, 0:8], mx[:, :], S[:, :])
                nc.vector.match_replace(out=S[:, :], in_to_replace=mx[:, :],
                                        in_values=S[:, :], imm_value=NEG_BIG)
                mx2 = sm_pool.tile([P, 8], F32, tag="mx2")
                nc.vector.max(out=mx2[:, :], in_=S[:, :])
                nc.vector.max_index(idx[:, 8:16], mx2[:, :], S[:, :])

                # ---- gather 16 bf16 db rows per point (one DMA per slot:
                # HW honors a single offset per partition) ----
                g = g_pool.tile([P, KNN * ROW], BF16)
                for nb in range(KNN):
                    nc.gpsimd.indirect_dma_start(
                        out=g[:, nb * ROW:(nb + 1) * ROW], out_offset=None,
                        in_=db_rows[:, :],
                        in_offset=bass.IndirectOffsetOnAxis(
                            ap=idx[:, nb:nb + 1], axis=0),
                    )

                # ---- transpose to channel-major, NEIGHBOR-major free
                # layout: G[cc][p, k*128 + j] (4 neighbors per PSUM tile,
                # contiguous copy out per group) ----
                G = [gt_pool.tile([P, P * KNN], BF16, tag=f"g{cc}",
                                  name=f"g{cc}") for cc in range(2)]
                P3 = gt_pool.tile([8, P * KNN], BF16, tag="p3")
                for grp in range(KNN // 4):
                    nbs = range(grp * 4, grp * 4 + 4)
                    gsl = slice(grp * 4 * P, (grp + 1) * 4 * P)
                    for cc in range(2):
                        tp = pp_tp.tile([P, 4 * P], BF16, tag="tp")
                        for q, nb in enumerate(nbs):
                            nc.tensor.transpose(
                                tp[:, q * P:(q + 1) * P],
                                g[:, nb * ROW + cc * P:nb * ROW + (cc + 1) * P],
                                ident[:, :])
                        nc.vector.tensor_copy(G[cc][:, gsl], tp[:, :])
                    tp2 = pp_tp.tile([8, 4 * P], BF16, tag="tp")
                    for q, nb in enumerate(nbs):
                        nc.tensor.transpose(
                            tp2[:, q * P:(q + 1) * P],
                            g[:, nb * ROW + 2 * P:nb * ROW + ROW],
                            ident[:, :])
                    nc.vector.tensor_copy(P3[:, gsl], tp2[:, :])

                # ---- MLPs in 512-wide slices (= 4 neighbors x 128 pts),
                # softmax + weighted sum at tile level ----
                out_t = [ot_pool.tile([P, P], F32, tag=f"ot{cc}",
                                      name=f"ot{cc}") for cc in range(2)]
                tpts = slice(t * P, (t + 1) * P)
                Vf = [tmp_pool.tile([P, P * KNN], BF16, tag=f"v{cc}",
                                    name=f"v{cc}") for cc in range(2)]
                ef = [tmp_pool.tile([P, P * KNN], BF16, tag=f"e{cc}",
                                    name=f"e{cc}") for cc in range(2)]
                for s in range(NSL):
                    sl = slice(s * SL, (s + 1) * SL)
                    # pos_rel = pcd_n - p_j  (rows 0-2; row-3 weight is 0)
                    pr = tmp_pool.tile([4, SL], BF16, tag="pr")
                    pcd_b = pcd_s[:, tpts].unsqueeze(1).to_broadcast(
                        [4, 4, P])
                    nc.vector.tensor_sub(
                        pr[:, :].rearrange("p (k j) -> p k j", j=P),
                        pcd_b, P3[0:4, sl].rearrange("p (k j) -> p k j", j=P))
                    # pos MLP
                    h1p_ps = pp_mlp.tile([PH, SL], F32, tag="mm")
                    nc.tensor.matmul(h1p_ps[:, :], lhsT=pw1[:, :], rhs=pr[:, :],
                                     start=True, stop=True)
                    h1p = tmp_pool.tile([PH, SL], BF16, tag="h1p")
                    nc.scalar.activation(h1p[:, :], h1p_ps[:, :], AF.Relu,
                                         bias=pb1[:, 0:1])
                    pe = [tmp_pool.tile([P, SL], BF16, tag=f"pe{cc}",
                                        name=f"pe{cc}") for cc in range(2)]
                    for cc in range(2):
                        pe_ps = pp_mlp.tile([P, SL], F32, tag="mm")
                        nc.tensor.matmul(pe_ps[:, :],
                                         lhsT=pw2[:, cc * P:(cc + 1) * P],
                                         rhs=h1p[:, :], start=True, stop=True)
                        nc.scalar.activation(pe[cc][:, :], pe_ps[:, :],
                                             AF.Copy)
                    # attn_in = feat_n - g + pos_emb ; V = g + pos_emb
                    ain = [tmp_pool.tile([P, SL], BF16, tag=f"ain{cc}",
                                         name=f"ain{cc}") for cc in range(2)]
                    for cc in range(2):
                        gsl = G[cc][:, sl]
                        nc.vector.tensor_sub(ain[cc][:, :], pe[cc][:, :], gsl)
                        featb = feat_s[cc][:, tpts].unsqueeze(1).to_broadcast(
                            [P, 4, P])
                        a3 = ain[cc][:, :].rearrange("p (k j) -> p k j", j=P)
                        nc.vector.tensor_add(a3, a3, featb)
                        nc.vector.tensor_add(Vf[cc][:, sl], pe[cc][:, :], gsl)
                    # attn MLP layer 1 (K=256 in 2 chunks, M=1024 in 8)
                    h1s = []
                    for o in range(AH // P):
                        hp = pp_mlp.tile([P, SL], F32, tag="mm")
                        nc.tensor.matmul(hp[:, :],
                                         lhsT=w1[0][:, o * P:(o + 1) * P],
                                         rhs=ain[0][:, :],
                                         start=True, stop=False)
                        nc.tensor.matmul(hp[:, :],
                                         lhsT=w1[1][:, o * P:(o + 1) * P],
                                         rhs=ain[1][:, :],
                                         start=False, stop=True)
                        ht = h1_pool.tile([P, SL], BF16, tag=f"h1_{o}",
                                          name=f"h1_{o}")
                        nc.scalar.activation(ht[:, :], hp[:, :], AF.Relu,
                                             bias=ab1[:, o:o + 1])
                        h1s.append(ht)
                    # attn MLP layer 2 (K=1024 in 8 chunks) + exp
                    for cc in range(2):
                        lp = pp_mlp.tile([P, SL], F32, tag="mm")
                        for o in range(AH // P):
                            nc.tensor.matmul(lp[:, :],
                                             lhsT=w2[o][:, cc * P:(cc + 1) * P],
                                             rhs=h1s[o][:, :],
                                             start=(o == 0),
                                             stop=(o == AH // P - 1))
                        nc.scalar.activation(ef[cc][:, sl], lp[:, :], AF.Exp,
                                             bias=ab2[:, cc:cc + 1])
                # ---- per-channel softmax over k + weighted sum (tile) ----
                for cc in range(2):
                    e3 = ef[cc][:, :].rearrange("p (k j) -> p j k", j=P)
                    den = sm_pool.tile([P, P], F32, tag=f"den{cc}",
                                       name=f"den{cc}")
                    nc.vector.reduce_sum(den[:, :], e3, axis=AX.X)
                    rden = sm_pool.tile([P, P], F32, tag=f"rden{cc}",
                                        name=f"rden{cc}")
                    nc.vector.reciprocal(rden[:, :], den[:, :])
                    ev = tmp_pool.tile([P, P * KNN], BF16, tag=f"ev{cc}",
                                       name=f"ev{cc}")
                    nc.vector.tensor_mul(ev[:, :], ef[cc][:, :], Vf[cc][:, :])
                    ev3 = ev[:, :].rearrange("p (k j) -> p j k", j=P)
                    num = sm_pool.tile([P, P], F32, tag=f"num{cc}",
                                       name=f"num{cc}")
                    nc.vector.reduce_sum(num[:, :], ev3, axis=AX.X)
                    nc.vector.tensor_mul(out_t[cc][:, :], num[:, :],
                                         rden[:, :])
                    # + pos_b2 (softmax weights sum to 1 per channel)
                    nc.vector.tensor_scalar(
                        out_t[cc][:, :], out_t[cc][:, :], pb2[:, cc:cc + 1],
                        None, op0=mybir.AluOpType.add)
                for cc in range(2):
                    nc.sync.dma_start(out[cc * P:(cc + 1) * P, tsl],
                                      out_t[cc][:, :])

    if legalize:
        _legalize_sync_waits(nc)
    return nc


_NC = None


def _get_nc():
    global _NC
    if _NC is None:
        _NC = _build_bass()
    return _NC


def _prep_in_maps(pcd, feat, pcd_feadb, feat_feadb,
                  pos_w1, pos_b1, pos_g1, pos_be1, pos_w2, pos_b2,
                  attn_w1, attn_b1, attn_g1, attn_be1, attn_w2, attn_b2):
    f32 = np.float32
    bf16 = mybir.dt.np(BF16)
    a = {k: np.ascontiguousarray(np.asarray(v), dtype=f32) for k, v in dict(
        pcd=pcd, feat=feat, pcd_feadb=pcd_feadb, feat_feadb=feat_feadb,
        pos_w1=pos_w1, pos_b1=pos_b1, pos_g1=pos_g1, pos_be1=pos_be1,
        pos_w2=pos_w2, pos_b2=pos_b2,
        attn_w1=attn_w1, attn_b1=attn_b1, attn_g1=attn_g1, attn_be1=attn_be1,
        attn_w2=attn_w2, attn_b2=attn_b2).items()}

    fus_pcd = np.concatenate([a['pcd'], a['pcd_feadb']], axis=2)    # [B,3,F]
    fus_feat = np.concatenate([a['feat'], a['feat_feadb']], axis=2)  # [B,C,F]

    # BatchNorm (eval, running stats 0/1) folded into the conv weights.
    sp = (a['pos_g1'].astype(np.float64) / np.sqrt(1.0 + BN_EPS))
    w1p = a['pos_w1'].astype(np.float64) * sp[:, None]
    b1p = a['pos_b1'].astype(np.float64) * sp + a['pos_be1']
    sa = (a['attn_g1'].astype(np.float64) / np.sqrt(1.0 + BN_EPS))
    w1a = a['attn_w1'].astype(np.float64) * sa[:, None]
    # pos_b2 folded into attn bias (pre-relu) and the final output bias.
    b1a = (a['attn_b1'].astype(np.float64) * sa + a['attn_be1']
           + w1a @ a['pos_b2'].astype(np.float64))

    pos_w1t = np.zeros((4, PH), bf16)
    pos_w1t[:3] = w1p.T.astype(bf16)
    pos_b1v = b1p.astype(f32).reshape(PH, 1)
    pos_w2t = np.ascontiguousarray(a['pos_w2'].T).astype(bf16)
    pos_b2c = np.ascontiguousarray(a['pos_b2'].reshape(2, P).T)
    attn_w1t = np.ascontiguousarray(w1a.T).astype(bf16)
    attn_b1c = np.ascontiguousarray(b1a.astype(f32).reshape(AH // P, P).T)
    attn_w2t = np.ascontiguousarray(a['attn_w2'].T).astype(bf16)
    attn_b2c = np.ascontiguousarray(a['attn_b2'].reshape(2, P).T)

    per_batch = []
    for b in range(B):
        knn_rhs = np.empty((4, F), f32)
        knn_rhs[:3] = fus_pcd[b]
        knn_rhs[3] = (-np.sum(fus_pcd[b].astype(np.float64) ** 2, axis=0)
                      ).astype(f32)
        db = np.zeros((F, ROW), bf16)
        db[:, :C] = fus_feat[b].T.astype(bf16)
        db[:, C:C + 3] = fus_pcd[b].T.astype(bf16)
        per_batch.append((knn_rhs, np.ascontiguousarray(db)))

    in_maps = []
    for core in range(NCORES):
        b, s = divmod(core, NCORES // B)
        sh = slice(s * SHARD, (s + 1) * SHARD)
        keys2t = np.zeros((4, SHARD), f32)
        keys2t[:3] = 2.0 * a['pcd'][b][:, sh]
        keys2t[3] = 1.0
        pcd_sh = np.zeros((4, SHARD), bf16)
        pcd_sh[:3] = a['pcd'][b][:, sh].astype(bf16)
        in_maps.append(dict(
            keys2t=keys2t,
            knn_rhs=per_batch[b][0],
            db_rows=per_batch[b][1],
            feat_sh=np.ascontiguousarray(a['feat'][b][:, sh]).astype(bf16),
            pcd_sh=pcd_sh,
            pos_w1t=pos_w1t, pos_b1=pos_b1v, pos_w2t=pos_w2t, pos_b2c=pos_b2c,
            attn_w1t=attn_w1t, attn_b1c=attn_b1c,
            attn_w2t=attn_w2t, attn_b2c=attn_b2c,
        ))
    return in_maps


def kernel(**inputs):
    global LAST_RESULT
    nc = _get_nc()
    in_maps = _prep_in_maps(**inputs)
    res = bass_utils.run_bass_kernel_spmd(
        nc, in_maps, core_ids=list(range(NCORES)), trace=TRACE)
    LAST_RESULT = res
    out = np.empty((B, C, N), np.float32)
    for core in range(NCORES):
        b, s = divmod(core, NCORES // B)
        out[b][:, s * SHARD:(s + 1) * SHARD] = res.results[core]["out"]
    return out


# revision 12
# speedup vs baseline: 1.1583x; 1.0476x over previous
"""CrossTransformer (KNN message passing) Trainium2 kernel.

Contract: kernel(**inputs) takes the FULL unsharded inputs (numpy arrays,
keys as in setup_inputs()) and returns the FULL [2, 256, 2048] float32
output.  Internally shards across 8 NeuronCores: core = b*4 + s handles
batch b, key-point shard s (512 points), with the fused KNN database
replicated per core.

Pipeline per core:
  1. KNN scores S = 2*k.f - |f|^2 via a K=4 fp32 matmul (PE) — kept fp32
     so the selected neighbor sets match the fp32 reference exactly;
     top-16 via DVE max/max_index/match_replace (two top-8 rounds).
  2. Indirect-DMA row gather of the bf16 [4096, 264] fused database
     (featT | pcdT | pad), one gather per neighbor slot (HW honors one
     offset per partition).
  3. PE transposes (4 neighbors packed per PSUM tile) to channel-major
     bf16 [256, points*16].
  4. pos/attn MLPs in bf16 with fp32 PSUM accumulation (BatchNorm folded
     into the weights host-side), exp without max-subtraction (logits are
     tiny), per-channel softmax over the 16 neighbors fused with the
     weighted sum; final reductions and output in fp32.
"""

import copy as _copy

import numpy as np

import concourse.bass as bass
import concourse.mybir as mybir
import concourse.tile as tile
from concourse import bass_utils
from concourse.masks import make_identity

F32 = mybir.dt.float32
BF16 = mybir.dt.bfloat16
U32 = mybir.dt.uint32
AF = mybir.ActivationFunctionType
AX = mybir.AxisListType

B = 2
C = 256
N = 2048
M = 2048
F = N + M            # fused database size
KNN = 16
PH = 64              # pos MLP hidden
AH = 1024            # attn MLP hidden
P = 128
NCORES = 8
SHARD = N * B // NCORES      # 512 key points per core
NT = SHARD // P              # 4 point-tiles per core
ROW = 264                    # db row: 256 feat + 3 pcd + 5 pad
SL = 512                     # free-dim slice (32 points x 16 neighbors)
PTS_SL = SL // KNN           # 32 points per slice
NSL = P * KNN // SL          # 4 slices per point-tile
BN_EPS = 1e-5
NEG_BIG = -3.0e38

# Module-level knobs for test harnesses (not used by the grader).
TRACE = False
LAST_RESULT = None

_NOP_DICT = {'header': {'opcode': 159, 'inst_word_len': 16}}


def _legalize_sync_waits(nc, max_waits=1):
    """walrus here accepts at most one sync wait per instruction; move
    extra waits onto ENGINE_NOP carriers inserted just before the offender
    (same engine: the sequencer accumulates the waits, no pipeline drain)."""
    module = nc.m
    new_module = _copy.replace(module, functions=[])
    for function in module.functions:
        new_function = _copy.replace(function, blocks=[])
        new_function.set_allocations_from_list(function.allocations)
        for block in function.blocks:
            out = []
            for inst in block.instructions:
                si = inst.sync_info
                waits = list(si.on_wait) if si is not None else []
                if len(waits) > max_waits:
                    extra, keep = waits[:-max_waits], waits[-max_waits:]
                    for j in range(0, len(extra), max_waits):
                        out.append(mybir.InstDrain(
                            name=f"I-lgl-{inst.name}-{j}",
                            engine=inst.engine,
                            ins=[], outs=[],
                            sync_info=mybir.SyncInfo(
                                on_wait=extra[j:j + max_waits], on_update=[]),
                        ))
                    inst.sync_info = mybir.SyncInfo(
                        on_wait=keep, on_update=list(si.on_update))
                out.append(inst)
            new_function.blocks.append(_copy.replace(block, instructions=out))
        new_module.functions.append(new_function)
    nc.m = new_module


def _build_bass(legalize=True):
    nc = bass.Bass()
    dt = nc.dram_tensor
    keys2t = dt("keys2t", [4, SHARD], F32, kind="ExternalInput")
    knn_rhs = dt("knn_rhs", [4, F], F32, kind="ExternalInput")
    db_rows = dt("db_rows", [F, ROW], BF16, kind="ExternalInput")
    feat_sh = dt("feat_sh", [C, SHARD], BF16, kind="ExternalInput")
    pcd_sh = dt("pcd_sh", [4, SHARD], BF16, kind="ExternalInput")
    pos_w1t = dt("pos_w1t", [4, PH], BF16, kind="ExternalInput")
    pos_b1 = dt("pos_b1", [PH, 1], F32, kind="ExternalInput")
    pos_w2t = dt("pos_w2t", [PH, C], BF16, kind="ExternalInput")
    pos_b2c = dt("pos_b2c", [P, 2], F32, kind="ExternalInput")
    attn_w1t = dt("attn_w1t", [C, AH], BF16, kind="ExternalInput")
    attn_b1c = dt("attn_b1c", [P, AH // P], F32, kind="ExternalInput")
    attn_w2t = dt("attn_w2t", [AH, C], BF16, kind="ExternalInput")
    attn_b2c = dt("attn_b2c", [P, 2], F32, kind="ExternalInput")
    out = dt("out", [C, SHARD], F32, kind="ExternalOutput")

    with tile.TileContext(nc) as tc:
        with (
            tc.tile_pool(name="const", bufs=1) as cp,
            tc.tile_pool(name="s", bufs=2) as s_pool,
            tc.tile_pool(name="idx", bufs=2) as idx_pool,
            tc.tile_pool(name="g", bufs=2) as g_pool,
            tc.tile_pool(name="gt", bufs=2) as gt_pool,
            tc.tile_pool(name="h1", bufs=1) as h1_pool,
            tc.tile_pool(name="tmp", bufs=2) as tmp_pool,
            tc.tile_pool(name="small", bufs=2) as sm_pool,
            tc.tile_pool(name="ot", bufs=2) as ot_pool,
            tc.tile_pool(name="ppk", bufs=1, space="PSUM") as pp_knn,
            tc.tile_pool(name="ppt", bufs=1, space="PSUM") as pp_tp,
            tc.tile_pool(name="ppm", bufs=2, space="PSUM") as pp_mlp,
            tc.tile_pool(name="ppp", bufs=2, space="PSUM") as pp_pos,
        ):
            # ---- constants / weights ----
            ident = cp.tile([P, P], BF16)
            make_identity(nc, ident[:, :])
            keys2t_s = cp.tile([4, SHARD], F32)
            nc.sync.dma_start(keys2t_s[:, :], keys2t[:, :])
            knn_rhs_s = cp.tile([4, F], F32)
            nc.sync.dma_start(knn_rhs_s[:, :], knn_rhs[:, :])
            feat_s = []
            for cc in range(2):
                ft = cp.tile([P, SHARD], BF16, tag=f"feat{cc}")
                nc.sync.dma_start(ft[:, :], feat_sh[cc * P:(cc + 1) * P, :])
                feat_s.append(ft)
            pcd_s = cp.tile([4, SHARD], BF16)
            nc.sync.dma_start(pcd_s[:, :], pcd_sh[:, :])
            pw1 = cp.tile([4, PH], BF16)
            nc.sync.dma_start(pw1[:, :], pos_w1t[:, :])
            pb1 = cp.tile([PH, 1], F32)
            nc.sync.dma_start(pb1[:, :], pos_b1[:, :])
            pw2 = cp.tile([PH, C], BF16)
            nc.sync.dma_start(pw2[:, :], pos_w2t[:, :])
            pb2 = cp.tile([P, 2], F32)
            nc.sync.dma_start(pb2[:, :], pos_b2c[:, :])
            w1 = []
            for kc in range(2):
                wt = cp.tile([P, AH], BF16, tag=f"w1_{kc}")
                nc.sync.dma_start(wt[:, :], attn_w1t[kc * P:(kc + 1) * P, :])
                w1.append(wt)
            ab1 = cp.tile([P, AH // P], F32)
            nc.sync.dma_start(ab1[:, :], attn_b1c[:, :])
            w2 = []
            for o in range(AH // P):
                wt = cp.tile([P, C], BF16, tag=f"w2_{o}")
                nc.sync.dma_start(wt[:, :], attn_w2t[o * P:(o + 1) * P, :])
                w2.append(wt)
            ab2 = cp.tile([P, 2], F32)
            nc.sync.dma_start(ab2[:, :], attn_b2c[:, :])

            for t in range(NT):
                tsl = slice(t * P, (t + 1) * P)
                # ---- KNN scores: S[p, f] = 2*k_p . f - |f|^2 (fp32) ----
                S = s_pool.tile([P, F], F32)
                for c in range(F // SL):
                    ps = pp_knn.tile([P, SL], F32, tag="ks")
                    nc.tensor.matmul(ps[:, :], lhsT=keys2t_s[:, tsl],
                                     rhs=knn_rhs_s[:, c * SL:(c + 1) * SL],
                                     start=True, stop=True)
                    nc.vector.tensor_copy(S[:, c * SL:(c + 1) * SL], ps[:, :])
                # ---- top-16 (two top-8 rounds; order within 16 is free) ----
                mx = sm_pool.tile([P, 8], F32, tag="mx")
                idx = idx_pool.tile([P, KNN], U32)
                nc.vector.max(out=mx[:, :], in_=S[:, :])
                nc.vector.max_index(idx[:, 0:8], mx[:, :], S[:, :])
                nc.vector.match_replace(out=S[:, :], in_to_replace=mx[:, :],
                                        in_values=S[:, :], imm_value=NEG_BIG)
                mx2 = sm_pool.tile([P, 8], F32, tag="mx2")
                nc.vector.max(out=mx2[:, :], in_=S[:, :])
                nc.vector.max_index(idx[:, 8:16], mx2[:, :], S[:, :])

                # ---- gather 16 bf16 db rows per point (one DMA per slot:
                # HW honors a single offset per partition) ----
                g = g_pool.tile([P, KNN * ROW], BF16)
                for nb in range(KNN):
                    nc.gpsimd.indirect_dma_start(
                        out=g[:, nb * ROW:(nb + 1) * ROW], out_offset=None,
                        in_=db_rows[:, :],
                        in_offset=bass.IndirectOffsetOnAxis(
                            ap=idx[:, nb:nb + 1], axis=0),
                    )

                # ---- transpose to channel-major, NEIGHBOR-major free
                # layout: G[cc][p, k*128 + j] (4 neighbors per PSUM tile,
                # contiguous copy out per group) ----
                G = [gt_pool.tile([P, P * KNN], BF16, tag=f"g{cc}",
                                  name=f"g{cc}") for cc in range(2)]
                P3 = gt_pool.tile([8, P * KNN], BF16, tag="p3")
                for grp in range(KNN // 4):
                    nbs = range(grp * 4, grp * 4 + 4)
                    gsl = slice(grp * 4 * P, (grp + 1) * 4 * P)
                    for cc in range(2):
                        tp = pp_tp.tile([P, 4 * P], BF16, tag="tp")
                        for q, nb in enumerate(nbs):
                            nc.tensor.transpose(
                                tp[:, q * P:(q + 1) * P],
                                g[:, nb * ROW + cc * P:nb * ROW + (cc + 1) * P],
                                ident[:, :])
                        nc.vector.tensor_copy(G[cc][:, gsl], tp[:, :])
                    tp2 = pp_tp.tile([8, 4 * P], BF16, tag="tp")
                    for q, nb in enumerate(nbs):
                        nc.tensor.transpose(
                            tp2[:, q * P:(q + 1) * P],
                            g[:, nb * ROW + 2 * P:nb * ROW + ROW],
                            ident[:, :])
                    nc.vector.tensor_copy(P3[:, gsl], tp2[:, :])

                # ---- MLPs: 512-wide slices (= 4 neighbors x 128 pts),
                # attn matmuls grouped in slice-pairs sharing a 2-bank PSUM
                # tile so relu/exp run 1024 wide ----
                out_t = [ot_pool.tile([P, P], F32, tag=f"ot{cc}",
                                      name=f"ot{cc}") for cc in range(2)]
                tpts = slice(t * P, (t + 1) * P)
                Vf = [tmp_pool.tile([P, P * KNN], BF16, tag=f"v{cc}",
                                    name=f"v{cc}") for cc in range(2)]
                ef = [tmp_pool.tile([P, P * KNN], BF16, tag=f"e{cc}",
                                    name=f"e{cc}") for cc in range(2)]
                for sp in range(NSL // 2):
                    ain = [[None, None], [None, None]]   # [sh][cc]
                    for sh in range(2):
                        s = sp * 2 + sh
                        sl = slice(s * SL, (s + 1) * SL)
                        # pos_rel = pcd_n - p_j (rows 0-2; row-3 weight 0)
                        pr = tmp_pool.tile([4, SL], BF16, tag="pr")
                        pcd_b = pcd_s[:, tpts].unsqueeze(1).to_broadcast(
                            [4, 4, P])
                        nc.vector.tensor_sub(
                            pr[:, :].rearrange("p (k j) -> p k j", j=P),
                            pcd_b,
                            P3[0:4, sl].rearrange("p (k j) -> p k j", j=P))
                        # pos MLP
                        h1p_ps = pp_pos.tile([PH, SL], F32, tag="mmp")
                        nc.tensor.matmul(h1p_ps[:, :], lhsT=pw1[:, :],
                                         rhs=pr[:, :], start=True, stop=True)
                        h1p = tmp_pool.tile([PH, SL], BF16, tag="h1p")
                        nc.scalar.activation(h1p[:, :], h1p_ps[:, :], AF.Relu,
                                             bias=pb1[:, 0:1])
                        pe = [tmp_pool.tile([P, SL], BF16, tag=f"pe{cc}",
                                            name=f"pe{cc}") for cc in range(2)]
                        for cc in range(2):
                            pe_ps = pp_pos.tile([P, SL], F32, tag="mmp")
                            nc.tensor.matmul(pe_ps[:, :],
                                             lhsT=pw2[:, cc * P:(cc + 1) * P],
                                             rhs=h1p[:, :],
                                             start=True, stop=True)
                            nc.scalar.activation(pe[cc][:, :], pe_ps[:, :],
                                                 AF.Copy)
                        # attn_in = feat_n - g + pos_emb ; V = g + pos_emb
                        for cc in range(2):
                            at = tmp_pool.tile([P, SL], BF16,
                                               tag=f"ain{sh}{cc}",
                                               name=f"ain{sh}{cc}")
                            gsl = G[cc][:, sl]
                            nc.vector.tensor_sub(at[:, :], pe[cc][:, :], gsl)
                            featb = feat_s[cc][:, tpts].unsqueeze(
                                1).to_broadcast([P, 4, P])
                            a3 = at[:, :].rearrange("p (k j) -> p k j", j=P)
                            nc.vector.tensor_add(a3, a3, featb)
                            nc.vector.tensor_add(Vf[cc][:, sl], pe[cc][:, :],
                                                 gsl)
                            ain[sh][cc] = at
                    # attn MLP layer 1: per output chunk, 4 matmuls share a
                    # 2-bank PSUM tile; one 1024-wide relu
                    h1s = []
                    for o in range(AH // P):
                        hp = pp_mlp.tile([P, 2 * SL], F32, tag="mm2")
                        for sh in range(2):
                            for kc in range(2):
                                nc.tensor.matmul(
                                    hp[:, sh * SL:(sh + 1) * SL],
                                    lhsT=w1[kc][:, o * P:(o + 1) * P],
                                    rhs=ain[sh][kc][:, :],
                                    start=(kc == 0), stop=(kc == 1))
                        ht = h1_pool.tile([P, 2 * SL], BF16, tag=f"h1_{o}",
                                          name=f"h1_{o}")
                        nc.scalar.activation(ht[:, :], hp[:, :], AF.Relu,
                                             bias=ab1[:, o:o + 1])
                        h1s.append(ht)
                    # attn MLP layer 2 + exp (1024 wide)
                    for cc in range(2):
                        lp = pp_mlp.tile([P, 2 * SL], F32, tag="mm2")
                        for o in range(AH // P):
                            for sh in range(2):
                                nc.tensor.matmul(
                                    lp[:, sh * SL:(sh + 1) * SL],
                                    lhsT=w2[o][:, cc * P:(cc + 1) * P],
                                    rhs=h1s[o][:, sh * SL:(sh + 1) * SL],
                                    start=(o == 0), stop=(o == AH // P - 1))
                        nc.scalar.activation(
                            ef[cc][:, sp * 2 * SL:(sp + 1) * 2 * SL],
                            lp[:, :], AF.Exp, bias=ab2[:, cc:cc + 1])
                # ---- per-channel softmax over k + weighted sum (tile).
                # k is the 128-periodic major dim, so sum over k = fold the
                # halves: contiguous adds, fp32 accumulation ----
                for cc in range(2):
                    def fold(src_ap, tag):
                        # [P, 16*128] -> [P, 128] summing the k-major halves
                        a = sm_pool.tile([P, 8 * P], F32, tag="folda",
                                         name="folda")
                        nc.vector.tensor_add(a[:, :], src_ap[:, :8 * P],
                                             src_ap[:, 8 * P:])
                        nc.vector.tensor_add(a[:, :4 * P], a[:, :4 * P],
                                             a[:, 4 * P:])
                        nc.vector.tensor_add(a[:, :2 * P], a[:, :2 * P],
                                             a[:, 2 * P:4 * P])
                        d = sm_pool.tile([P, P], F32, tag=f"{tag}d",
                                         name=f"{tag}d")
                        nc.vector.tensor_add(d[:, :], a[:, :P], a[:, P:2 * P])
                        return d
                    den = fold(ef[cc], f"den{cc}")
                    rden = sm_pool.tile([P, P], F32, tag=f"rden{cc}",
                                        name=f"rden{cc}")
                    nc.vector.reciprocal(rden[:, :], den[:, :])
                    ev = tmp_pool.tile([P, P * KNN], BF16, tag="ev",
                                       name="ev")
                    nc.vector.tensor_mul(ev[:, :], ef[cc][:, :], Vf[cc][:, :])
                    num = fold(ev, f"num{cc}")
                    nc.vector.tensor_mul(out_t[cc][:, :], num[:, :],
                                         rden[:, :])
                    # + pos_b2 (softmax weights sum to 1 per channel)
                    nc.vector.tensor_scalar(
                        out_t[cc][:, :], out_t[cc][:, :], pb2[:, cc:cc + 1],
                        None, op0=mybir.AluOpType.add)
                for cc in range(2):
                    nc.sync.dma_start(out[cc * P:(cc + 1) * P, tsl],
                                      out_t[cc][:, :])

    if legalize:
        _legalize_sync_waits(nc)
    return nc


_NC = None


def _get_nc():
    global _NC
    if _NC is None:
        _NC = _build_bass()
    return _NC


def _prep_in_maps(pcd, feat, pcd_feadb, feat_feadb,
                  pos_w1, pos_b1, pos_g1, pos_be1, pos_w2, pos_b2,
                  attn_w1, attn_b1, attn_g1, attn_be1, attn_w2, attn_b2):
    f32 = np.float32
    bf16 = mybir.dt.np(BF16)
    a = {k: np.ascontiguousarray(np.asarray(v), dtype=f32) for k, v in dict(
        pcd=pcd, feat=feat, pcd_feadb=pcd_feadb, feat_feadb=feat_feadb,
        pos_w1=pos_w1, pos_b1=pos_b1, pos_g1=pos_g1, pos_be1=pos_be1,
        pos_w2=pos_w2, pos_b2=pos_b2,
        attn_w1=attn_w1, attn_b1=attn_b1, attn_g1=attn_g1, attn_be1=attn_be1,
        attn_w2=attn_w2, attn_b2=attn_b2).items()}

    fus_pcd = np.concatenate([a['pcd'], a['pcd_feadb']], axis=2)    # [B,3,F]
    fus_feat = np.concatenate([a['feat'], a['feat_feadb']], axis=2)  # [B,C,F]

    # BatchNorm (eval, running stats 0/1) folded into the conv weights.
    sp = (a['pos_g1'].astype(np.float64) / np.sqrt(1.0 + BN_EPS))
    w1p = a['pos_w1'].astype(np.float64) * sp[:, None]
    b1p = a['pos_b1'].astype(np.float64) * sp + a['pos_be1']
    sa = (a['attn_g1'].astype(np.float64) / np.sqrt(1.0 + BN_EPS))
    w1a = a['attn_w1'].astype(np.float64) * sa[:, None]
    # pos_b2 folded into attn bias (pre-relu) and the final output bias.
    b1a = (a['attn_b1'].astype(np.float64) * sa + a['attn_be1']
           + w1a @ a['pos_b2'].astype(np.float64))

    pos_w1t = np.zeros((4, PH), bf16)
    pos_w1t[:3] = w1p.T.astype(bf16)
    pos_b1v = b1p.astype(f32).reshape(PH, 1)
    pos_w2t = np.ascontiguousarray(a['pos_w2'].T).astype(bf16)
    pos_b2c = np.ascontiguousarray(a['pos_b2'].reshape(2, P).T)
    attn_w1t = np.ascontiguousarray(w1a.T).astype(bf16)
    attn_b1c = np.ascontiguousarray(b1a.astype(f32).reshape(AH // P, P).T)
    attn_w2t = np.ascontiguousarray(a['attn_w2'].T).astype(bf16)
    attn_b2c = np.ascontiguousarray(a['attn_b2'].reshape(2, P).T)

    per_batch = []
    for b in range(B):
        knn_rhs = np.empty((4, F), f32)
        knn_rhs[:3] = fus_pcd[b]
        knn_rhs[3] = (-np.sum(fus_pcd[b].astype(np.float64) ** 2, axis=0)
                      ).astype(f32)
        db = np.zeros((F, ROW), bf16)
        db[:, :C] = fus_feat[b].T.astype(bf16)
        db[:, C:C + 3] = fus_pcd[b].T.astype(bf16)
        per_batch.append((knn_rhs, np.ascontiguousarray(db)))

    in_maps = []
    for core in range(NCORES):
        b, s = divmod(core, NCORES // B)
        sh = slice(s * SHARD, (s + 1) * SHARD)
        keys2t = np.zeros((4, SHARD), f32)
        keys2t[:3] = 2.0 * a['pcd'][b][:, sh]
        keys2t[3] = 1.0
        pcd_sh = np.zeros((4, SHARD), bf16)
        pcd_sh[:3] = a['pcd'][b][:, sh].astype(bf16)
        in_maps.append(dict(
            keys2t=keys2t,
            knn_rhs=per_batch[b][0],
            db_rows=per_batch[b][1],
            feat_sh=np.ascontiguousarray(a['feat'][b][:, sh]).astype(bf16),
            pcd_sh=pcd_sh,
            pos_w1t=pos_w1t, pos_b1=pos_b1v, pos_w2t=pos_w2t, pos_b2c=pos_b2c,
            attn_w1t=attn_w1t, attn_b1c=attn_b1c,
            attn_w2t=attn_w2t, attn_b2c=attn_b2c,
        ))
    return in_maps


def kernel(**inputs):
    global LAST_RESULT
    nc = _get_nc()
    in_maps = _prep_in_maps(**inputs)
    res = bass_utils.run_bass_kernel_spmd(
        nc, in_maps, core_ids=list(range(NCORES)), trace=TRACE)
    LAST_RESULT = res
    out = np.empty((B, C, N), np.float32)
    for core in range(NCORES):
        b, s = divmod(core, NCORES // B)
        out[b][:, s * SHARD:(s + 1) * SHARD] = res.results[core]["out"]
    return out
